# revision 32
# baseline (speedup 1.0000x reference)
"""Distributed causal multi-head attention kernel for 8 TRN2 NeuronCores.

Problem: B=2, S=2048, D=1024, H=16 heads (hd=64), f32 I/O, causal softmax.
Sharding: data-parallel over batch (2 groups of 4 cores), tensor-parallel over
heads within each group (4 heads/core) and over wo output rows.

Fused-pipeline design (engine queues are in-order, so overlap is created by
program-order interleaving):
  - projections of chunk c+1 and wo-matmuls of gathered chunks are emitted as
    "filler" PE groups between attention m-iterations of chunk c, keeping the
    PE dense (HAM stays warm) while the ACT engine works through the exps
  - yT is AllGathered per 512-col q-chunk the moment it is normalized, so only
    the last chunk's AG is (partially) exposed
  - weight DMAs are split per 128-row k-tile and spread across the tensor/
    scalar/vector engine DMA queues so the first matmul can start ~1us in
  - PSUM: pj pool (2 banks, proj+wo) + sc (2 banks) + av (4 banks) = 8
"""
import numpy as np
import ml_dtypes

import concourse.bass as bass
import concourse.bacc as bacc
import concourse.tile as tile
from concourse import mybir
from concourse.bass import ts

B, S, D, H = 2, 2048, 1024, 16
HD = D // H            # 64
N_CORES = 8
TP = 4                 # cores per batch group
HPC = H // TP          # heads per core = 4
DPC = D // TP          # 256: head-dims per core, also wo dout shard
GROUPS = [[0, 1, 2, 3], [4, 5, 6, 7]]
QC = 512               # q-chunk (free dim of scores), also AG granularity
KT = 128               # k-tile (partition dim of scores)
NQC = S // QC          # 4
NKT = S // KT          # 16
NK = D // 128          # 8 contraction tiles for projections

BF16 = mybir.dt.bfloat16
F32 = mybir.dt.float32


def build():
    nc = bacc.Bacc(None, target_bir_lowering=False, debug=False)

    xT = nc.declare_dram_parameter("xT", [D, S], BF16, isOutput=False)
    wqT = nc.declare_dram_parameter("wqT", [D, DPC], BF16, isOutput=False)
    wkT = nc.declare_dram_parameter("wkT", [D, DPC], BF16, isOutput=False)
    wvT = nc.declare_dram_parameter("wvT", [D, DPC], BF16, isOutput=False)
    woT = nc.declare_dram_parameter("woT", [D, DPC], BF16, isOutput=False)
    masks = nc.declare_dram_parameter("masks", [4, KT, 2 * QC], BF16, isOutput=False)
    out = nc.declare_dram_parameter("out", [DPC, S], F32, isOutput=True)

    with tile.TileContext(nc) as tc:
        with (
            tc.tile_pool(name="persist", bufs=1) as persist,
            tc.tile_pool(name="xtp", bufs=1) as xtp,
            tc.tile_pool(name="ptile", bufs=4) as ptile,
            tc.tile_pool(name="norm", bufs=4) as norm,
            tc.tile_pool(name="gqp", bufs=1) as gqp,
            tc.tile_pool(name="dram", bufs=1, space="DRAM") as dram,
            tc.tile_pool(name="pj_ps", bufs=2, space="PSUM") as pj_ps,
            tc.tile_pool(name="sc_ps", bufs=2, space="PSUM") as sc_ps,
            tc.tile_pool(name="av_ps", bufs=1, space="PSUM") as av_ps,
        ):
            # ---- persistent SBUF ----
            xt3 = xtp.tile([128, NK, S], BF16, name="xt3")
            xt = [xt3[:, k, :] for k in range(NK)]
            wq_s3 = persist.tile([128, NK, DPC], BF16, name="wq_s3")
            wk_s3 = persist.tile([128, NK, DPC], BF16, name="wk_s3")
            wv_s3 = persist.tile([128, NK, DPC], BF16, name="wv_s3")
            wo_s3 = persist.tile([128, NK, DPC], BF16, name="wo_s3")
            wq_s = [wq_s3[:, k, :] for k in range(NK)]
            wk_s = [wk_s3[:, k, :] for k in range(NK)]
            wv_s = [wv_s3[:, k, :] for k in range(NK)]
            wo_s = [wo_s3[:, k, :] for k in range(NK)]
            mask_t = [persist.tile([KT, 2 * QC], BF16, name=f"mask{r}") for r in range(4)]
            qT = [persist.tile([128, S], BF16, name=f"qT{hp}") for hp in range(2)]
            kT = [persist.tile([128, S], BF16, name=f"kT{hp}") for hp in range(2)]
            vt = [persist.tile([128, HPC, HD + 1], BF16, name=f"v{st}") for st in range(NKT)]
            yT = [persist.tile([HD, S], BF16, name=f"yT{h}") for h in range(HPC)]
            gq = [
                [gqp.tile([128, QC], BF16, name=f"gq{c}_{k}") for k in range(NK)]
                for c in range(NQC)
            ]

            # whole-tensor weight loads, spread across queues: wq on the
            # scalar queue (first matmul needs it), the rest on gpsimd.
            # x is loaded chunk-0 columns first so proj(0) never waits on a
            # whole-row DMA mid-flight (those waits kept HAM cold in v4)
            nc.scalar.dma_start(
                out=wq_s3, in_=wqT.rearrange("(k p) d -> p k d", p=128)
            )
            nc.gpsimd.dma_start(
                out=wk_s3, in_=wkT.rearrange("(k p) d -> p k d", p=128)
            )
            nc.gpsimd.dma_start(
                out=wv_s3, in_=wvT.rearrange("(k p) d -> p k d", p=128)
            )
            for k in range(NK):
                nc.sync.dma_start(out=xt[k][:, 0:QC], in_=xT[ts(k, 128), 0:QC])
            for k in range(NK):
                nc.sync.dma_start(out=xt[k][:, QC:S], in_=xT[ts(k, 128), QC:S])
            for r in range(4):
                nc.gpsimd.dma_start(out=mask_t[r], in_=masks[r])
            nc.gpsimd.dma_start(
                out=wo_s3, in_=woT.rearrange("(k p) d -> p k d", p=128)
            )

            # warm up the ACT exp table during the DMA preamble so the first
            # real exp doesn't pay the ~1.3us ACT_TABLE_LOAD
            warm = persist.tile([128, 16], F32, name="warm")
            nc.vector.memset(warm, 0.0)
            warm2 = persist.tile([128, 16], F32, name="warm2")
            nc.scalar.activation(
                warm2, warm, mybir.ActivationFunctionType.Exp, scale=1.0
            )

            # softmax-denominator ones column of every v tile, written once
            for st in range(NKT):
                nc.vector.memset(vt[st][:, :, HD:HD + 1], 1.0)

            gath = [None] * NQC

            # ---------- PE work groups ----------
            # qk/wo groups are emitted in two 4-matmul halves so a filler
            # never inserts more than ~1.1us between attention score pairs
            pj_half = {}

            def emit_qk(c, which, m, half=None):
                w_s = wq_s if which == "q" else wk_s
                dst = qT if which == "q" else kT
                key = (c, which, m)
                if half in (None, 0):
                    pj_half[key] = pj_ps.tile(
                        [128, QC], F32, tag="pj", name=f"{which}p{c}_{m}"
                    )
                p = pj_half[key]
                ks = range(NK) if half is None else (
                    range(NK // 2) if half == 0 else range(NK // 2, NK)
                )
                for k in ks:
                    nc.tensor.matmul(
                        p, w_s[k][:, ts(m, 128)], xt[k][:, ts(c, QC)],
                        start=(k == 0), stop=(k == NK - 1),
                    )
                if half in (None, 1):
                    nc.vector.tensor_copy(dst[m][:, ts(c, QC)], p)

            def emit_v(c, sm):
                st = c * 4 + sm
                p = pj_ps.tile([128, QC], F32, tag="pj", name=f"vp{st}")
                for k in range(NK):
                    nc.tensor.matmul(
                        p[:, 0:DPC], xt[k][:, ts(st, 128)], wv_s[k],
                        start=(k == 0), stop=(k == NK - 1),
                    )
                nc.vector.tensor_copy(
                    vt[st][:, :, 0:HD],
                    p[:, 0:DPC].rearrange("p (h d) -> p h d", h=HPC),
                )

            def emit_wo(c, m, half=None):
                key = (c, "wo", m)
                if half in (None, 0):
                    pj_half[key] = pj_ps.tile(
                        [128, QC], F32, tag="pj", name=f"wp{c}_{m}"
                    )
                p = pj_half[key]
                ks = range(NK) if half is None else (
                    range(NK // 2) if half == 0 else range(NK // 2, NK)
                )
                for k in ks:
                    nc.tensor.matmul(
                        p, wo_s[k][:, ts(m, 128)], gq[c][k],
                        start=(k == 0), stop=(k == NK - 1),
                    )
                if half in (None, 1):
                    ow = norm.tile([128, QC], F32, tag="ow", name=f"ow{c}_{m}")
                    nc.vector.tensor_copy(ow, p)
                    nc.sync.dma_start(out=out[ts(m, 128), ts(c, QC)], in_=ow)

            # filler assignment: PE groups interleaved into attn(c)'s stream,
            # keyed by the m-iteration index at whose TOP they are emitted.
            # v(c+?) groups must precede the AV matmuls that read their vt
            # tiles; wo(c) groups must come well after AG(c) is in flight.
            fillers = {
                0: {0: ("v", 0, 0), 1: ("v", 0, 1), 2: ("v", 0, 2), 3: ("v", 0, 3),
                    4: ("qk", 1, "q", 0), 5: ("qk", 1, "q", 1),
                    6: ("qk", 1, "k", 0), 7: ("qk", 1, "k", 1)},
                1: {0: ("v", 1, 0), 1: ("v", 1, 1), 2: ("v", 1, 2), 3: ("v", 1, 3),
                    6: ("qk", 2, "q", 0), 9: ("qk", 2, "q", 1),
                    12: ("qk", 2, "k", 0), 15: ("qk", 2, "k", 1)},
                2: {0: ("v", 2, 0), 1: ("v", 2, 1), 2: ("v", 2, 2), 3: ("v", 2, 3),
                    6: ("qk", 3, "q", 0), 10: ("qk", 3, "q", 1),
                    14: ("qk", 3, "k", 0), 16: ("qk", 3, "k", 1),
                    18: ("wo", 0, 0), 21: ("wo", 0, 1)},
                3: {0: ("v", 3, 0), 1: ("v", 3, 1), 2: ("v", 3, 2), 3: ("v", 3, 3),
                    6: ("wo", 1, 0), 10: ("wo", 1, 1),
                    28: ("wo", 2, 0), 30: ("wo", 2, 1)},
            }

            def emit_filler(f):
                if f[0] == "qk":
                    emit_qk(f[1], f[2], f[3])
                elif f[0] == "v":
                    emit_v(f[1], f[2])
                else:
                    emit_wo(f[1], f[2])

            def attn_chunk(qc):
                fl = dict(fillers[qc])
                n_k = (qc + 1) * 4
                it = 0
                bounce = dram.tile([DPC, QC], BF16, name=f"bounce{qc}")
                for hp in range(2):
                    avA = av_ps.tile([128, QC], F32, tag="avA", name=f"avA{qc}_{hp}")
                    avB = av_ps.tile([128, QC], F32, tag="avB", name=f"avB{qc}_{hp}")
                    for m in range(n_k):
                        if it in fl:
                            emit_filler(fl.pop(it))
                        it += 1
                        # causal: columns j < off are fully masked for this k-tile
                        off = max(0, (m - 4 * qc) * 128)
                        sc = sc_ps.tile(
                            [128, 2 * QC], F32, tag="sc", name=f"sc{qc}_{hp}_{m}"
                        )
                        sc3 = sc[:].rearrange("p (t q) -> p t q", t=2)
                        nc.tensor.matmul(
                            sc[:, off:QC],
                            kT[hp][0:64, ts(m, 128)],
                            qT[hp][0:64, qc * QC + off:(qc + 1) * QC],
                            start=True, stop=True,
                        )
                        nc.tensor.matmul(
                            sc[:, QC + off:2 * QC],
                            kT[hp][64:128, ts(m, 128)],
                            qT[hp][64:128, qc * QC + off:(qc + 1) * QC],
                            start=True, stop=True,
                        )
                        pt = ptile.tile(
                            [128, 2 * QC], BF16, tag="pt", name=f"pt{qc}_{hp}_{m}"
                        )
                        pt3 = pt[:].rearrange("p (t q) -> p t q", t=2)
                        nc.scalar.activation(
                            pt3[:, :, off:QC], sc3[:, :, off:QC],
                            mybir.ActivationFunctionType.Exp,
                            scale=1.0 / np.sqrt(HD),
                        )
                        if m >= 4 * qc:
                            ri = m - 4 * qc
                            m3 = mask_t[ri][:].rearrange("p (t q) -> p t q", t=2)
                            nc.vector.tensor_mul(
                                pt3[:, :, off:QC], pt3[:, :, off:QC],
                                m3[:, :, off:QC],
                            )
                        nc.tensor.matmul(
                            avA[0:HD + 1, off:QC], vt[m][:, 2 * hp, :],
                            pt[:, off:QC],
                            start=(m == 0), stop=(m == n_k - 1),
                        )
                        nc.tensor.matmul(
                            avB[0:HD + 1, off:QC], vt[m][:, 2 * hp + 1, :],
                            pt[:, QC + off:2 * QC],
                            start=(m == 0), stop=(m == n_k - 1),
                        )
                    for hh, av in ((2 * hp, avA), (2 * hp + 1, avB)):
                        # drain av (PSUM) quickly so the next hp's AV can
                        # reuse the bank: rows 0:64 to avs, denom row to a
                        # partition-0 tile for the (partition-aligned) recip
                        avs = norm.tile([HD, QC], F32, tag="avs", name=f"avs{qc}_{hh}")
                        nc.vector.tensor_copy(avs, av[0:HD, :])
                        dn = norm.tile([1, QC], F32, tag="dn", name=f"dn{qc}_{hh}")
                        nc.vector.tensor_copy(dn, av[HD:HD + 1, :])
                        recip = norm.tile([1, QC], F32, tag="recip", name=f"rc{qc}_{hh}")
                        nc.vector.reciprocal_approx_fast(recip, dn)
                        rb = norm.tile([HD, QC], F32, tag="rb", name=f"rb{qc}_{hh}")
                        nc.gpsimd.partition_broadcast(rb, recip, channels=HD)
                        nc.vector.tensor_mul(
                            yT[hh][:, ts(qc, QC)], avs, rb
                        )
                # leftover fillers (shouldn't normally happen)
                for key in sorted(fl):
                    emit_filler(fl.pop(key))

                # ship this chunk's yT through the group AllGather immediately
                for h in range(HPC):
                    nc.sync.dma_start(
                        out=bounce[ts(h, HD), :], in_=yT[h][:, ts(qc, QC)]
                    )
                g = dram.tile([D, QC], BF16, name=f"gath{qc}")
                nc.gpsimd.collective_compute(
                    "AllGather",
                    mybir.AluOpType.bypass,
                    replica_groups=GROUPS,
                    ins=[bounce[:].opt()],
                    outs=[g[:].opt()],
                )
                gath[qc] = g

            def emit_gq_loads(c):
                for k in range(NK):
                    nc.sync.dma_start(out=gq[c][k], in_=gath[c][ts(k, 128), :])

            # ---------- fused pipeline ----------
            for m in range(2):
                emit_qk(0, "q", m)
            for m in range(2):
                emit_qk(0, "k", m)

            attn_chunk(0)
            attn_chunk(1)
            emit_gq_loads(0)        # AG0 done by now; must not block bounce(2)
            attn_chunk(2)
            emit_gq_loads(1)        # AG1 long done
            emit_gq_loads(2)        # waits on AG2 (in flight) on the sync queue
            attn_chunk(3)
            emit_gq_loads(3)        # waits on AG3; nothing left on sync queue
            emit_wo(3, 0)
            emit_wo(3, 1)

    nc.finalize()
    return nc


def make_masks():
    i = np.arange(KT)[:, None]
    j = np.arange(QC)[None, :]
    m = np.zeros((4, KT, 2 * QC), dtype=ml_dtypes.bfloat16)
    for r in range(4):
        half = ((r * KT + i) <= j).astype(ml_dtypes.bfloat16)
        m[r, :, 0:QC] = half
        m[r, :, QC:2 * QC] = half
    return m


def shard_inputs(x, wq, wk, wv, wo):
    """Full f32 inputs -> per-core in_maps (bf16)."""
    bf = ml_dtypes.bfloat16
    masks = make_masks()
    wqT = np.ascontiguousarray(wq.T).astype(bf)
    wkT = np.ascontiguousarray(wk.T).astype(bf)
    wvT = np.ascontiguousarray(wv.T).astype(bf)
    woT = np.ascontiguousarray(wo.T).astype(bf)
    in_maps = []
    for c in range(N_CORES):
        b, tp = divmod(c, TP)
        sl = slice(tp * DPC, (tp + 1) * DPC)
        in_maps.append({
            "xT": np.ascontiguousarray(x[b].T).astype(bf),
            "wqT": np.ascontiguousarray(wqT[:, sl]),
            "wkT": np.ascontiguousarray(wkT[:, sl]),
            "wvT": np.ascontiguousarray(wvT[:, sl]),
            "woT": np.ascontiguousarray(woT[:, sl]),
            "masks": masks,
        })
    return in_maps


def assemble_output(results):
    """Per-core F^T shards [DPC, S] -> full [B, S, D] f32."""
    outs = []
    for b in range(B):
        ft = np.concatenate(
            [results[b * TP + tp]["out"] for tp in range(TP)], axis=0
        )  # [D, S]
        outs.append(ft.T)  # [S, D]
    return np.stack(outs, axis=0)


_NC_CACHE = []


def kernel(x, wq, wk, wv, wo):
    """Full-input distributed attention on 8 NeuronCores; returns full output."""
    x = np.asarray(x, dtype=np.float32)
    wq = np.asarray(wq, dtype=np.float32)
    wk = np.asarray(wk, dtype=np.float32)
    wv = np.asarray(wv, dtype=np.float32)
    wo = np.asarray(wo, dtype=np.float32)
    if not _NC_CACHE:
        _NC_CACHE.append(build())
    nc = _NC_CACHE[0]
    in_maps = shard_inputs(x, wq, wk, wv, wo)
    from concourse import bass2jax
    results = bass2jax.run_bass_via_pjrt(nc, in_maps, n_cores=N_CORES)
    return assemble_output(results).astype(np.float32)


# revision 33
# speedup vs baseline: 1.1124x; 1.1124x over previous
"""Distributed causal multi-head attention kernel for 8 TRN2 NeuronCores.

Problem: B=2, S=2048, D=1024, H=16 heads (hd=64), f32 I/O, causal softmax.
Sharding: data-parallel over batch (2 groups of 4 cores), tensor-parallel over
heads within each group (4 heads/core) and over wo output rows.

Fused-pipeline design (engine queues are in-order, so overlap is created by
program-order interleaving):
  - projections of chunk c+1 and wo-matmuls of gathered chunks are emitted as
    "filler" PE groups between attention m-iterations of chunk c, keeping the
    PE dense (HAM stays warm) while the ACT engine works through the exps
  - yT is AllGathered per 512-col q-chunk the moment it is normalized, so only
    the last chunk's AG is (partially) exposed
  - weight DMAs are split per 128-row k-tile and spread across the tensor/
    scalar/vector engine DMA queues so the first matmul can start ~1us in
  - PSUM: pj pool (2 banks, proj+wo) + sc (2 banks) + av (4 banks) = 8
"""
import numpy as np
import ml_dtypes

import concourse.bass as bass
import concourse.bacc as bacc
import concourse.tile as tile
from concourse import mybir
from concourse.bass import ts

B, S, D, H = 2, 2048, 1024, 16
HD = D // H            # 64
N_CORES = 8
TP = 4                 # cores per batch group
HPC = H // TP          # heads per core = 4
DPC = D // TP          # 256: head-dims per core, also wo dout shard
GROUPS = [[0, 1, 2, 3], [4, 5, 6, 7]]
QC = 512               # q-chunk (free dim of scores), also AG granularity
KT = 128               # k-tile (partition dim of scores)
NQC = S // QC          # 4
NKT = S // KT          # 16
NK = D // 128          # 8 contraction tiles for projections

BF16 = mybir.dt.bfloat16
F32 = mybir.dt.float32


def build():
    nc = bacc.Bacc(None, target_bir_lowering=False, debug=False)

    xT = nc.declare_dram_parameter("xT", [D, S], BF16, isOutput=False)
    wqT = nc.declare_dram_parameter("wqT", [D, DPC], BF16, isOutput=False)
    wkT = nc.declare_dram_parameter("wkT", [D, DPC], BF16, isOutput=False)
    wvT = nc.declare_dram_parameter("wvT", [D, DPC], BF16, isOutput=False)
    woT = nc.declare_dram_parameter("woT", [D, DPC], BF16, isOutput=False)
    masks = nc.declare_dram_parameter("masks", [4, KT, 2 * QC], BF16, isOutput=False)
    out = nc.declare_dram_parameter("out", [DPC, S], F32, isOutput=True)

    with tile.TileContext(nc) as tc:
        with (
            tc.tile_pool(name="persist", bufs=1) as persist,
            tc.tile_pool(name="xtp", bufs=1) as xtp,
            tc.tile_pool(name="ptile", bufs=4) as ptile,
            tc.tile_pool(name="norm", bufs=4) as norm,
            tc.tile_pool(name="gqp", bufs=1) as gqp,
            tc.tile_pool(name="dram", bufs=1, space="DRAM") as dram,
            tc.tile_pool(name="pj_ps", bufs=2, space="PSUM") as pj_ps,
            tc.tile_pool(name="sc_ps", bufs=2, space="PSUM") as sc_ps,
            tc.tile_pool(name="av_ps", bufs=1, space="PSUM") as av_ps,
        ):
            # ---- persistent SBUF ----
            xt = [xtp.tile([128, S], BF16, name=f"xt{k}") for k in range(NK)]
            wq_s3 = persist.tile([128, NK, DPC], BF16, name="wq_s3")
            wk_s3 = persist.tile([128, NK, DPC], BF16, name="wk_s3")
            wv_s3 = persist.tile([128, NK, DPC], BF16, name="wv_s3")
            wo_s3 = persist.tile([128, NK, DPC], BF16, name="wo_s3")
            wq_s = [wq_s3[:, k, :] for k in range(NK)]
            wk_s = [wk_s3[:, k, :] for k in range(NK)]
            wv_s = [wv_s3[:, k, :] for k in range(NK)]
            wo_s = [wo_s3[:, k, :] for k in range(NK)]
            mask_t = [persist.tile([KT, 2 * QC], BF16, name=f"mask{r}") for r in range(4)]
            qT = [persist.tile([128, S], BF16, name=f"qT{hp}") for hp in range(2)]
            kT = [persist.tile([128, S], BF16, name=f"kT{hp}") for hp in range(2)]
            vt = [persist.tile([128, HPC, HD + 1], BF16, name=f"v{st}") for st in range(NKT)]
            yT = [persist.tile([HD, S], BF16, name=f"yT{h}") for h in range(HPC)]
            gq = [
                [gqp.tile([128, QC], BF16, name=f"gq{c}_{k}") for k in range(NK)]
                for c in range(NQC)
            ]

            # whole-tensor weight loads, spread across queues: wq on the
            # scalar queue (first matmul needs it), the rest on gpsimd.
            # x is loaded chunk-0 columns first so proj(0) never waits on a
            # whole-row DMA mid-flight (those waits kept HAM cold in v4)
            nc.scalar.dma_start(
                out=wq_s3, in_=wqT.rearrange("(k p) d -> p k d", p=128)
            )
            nc.gpsimd.dma_start(
                out=wk_s3, in_=wkT.rearrange("(k p) d -> p k d", p=128)
            )
            nc.gpsimd.dma_start(
                out=wv_s3, in_=wvT.rearrange("(k p) d -> p k d", p=128)
            )
            for k in range(NK):
                nc.sync.dma_start(out=xt[k][:, 0:QC], in_=xT[ts(k, 128), 0:QC])
            for k in range(NK):
                nc.sync.dma_start(out=xt[k][:, QC:S], in_=xT[ts(k, 128), QC:S])
            for r in range(4):
                nc.gpsimd.dma_start(out=mask_t[r], in_=masks[r])
            nc.gpsimd.dma_start(
                out=wo_s3, in_=woT.rearrange("(k p) d -> p k d", p=128)
            )

            # warm up the ACT exp table during the DMA preamble so the first
            # real exp doesn't pay the ~1.3us ACT_TABLE_LOAD
            warm = persist.tile([128, 16], F32, name="warm")
            nc.vector.memset(warm, 0.0)
            warm2 = persist.tile([128, 16], F32, name="warm2")
            nc.scalar.activation(
                warm2, warm, mybir.ActivationFunctionType.Exp, scale=1.0
            )

            # softmax-denominator ones column of every v tile, written once
            for st in range(NKT):
                nc.vector.memset(vt[st][:, :, HD:HD + 1], 1.0)

            gath = [None] * NQC

            # ---------- PE work groups ----------
            # qk/wo groups are emitted in two 4-matmul halves so a filler
            # never inserts more than ~1.1us between attention score pairs
            pj_half = {}

            def emit_qk(c, which, m, half=None):
                w_s = wq_s if which == "q" else wk_s
                dst = qT if which == "q" else kT
                key = (c, which, m)
                if half in (None, 0):
                    pj_half[key] = pj_ps.tile(
                        [128, QC], F32, tag="pj", name=f"{which}p{c}_{m}"
                    )
                p = pj_half[key]
                ks = range(NK) if half is None else (
                    range(NK // 2) if half == 0 else range(NK // 2, NK)
                )
                for k in ks:
                    nc.tensor.matmul(
                        p, w_s[k][:, ts(m, 128)], xt[k][:, ts(c, QC)],
                        start=(k == 0), stop=(k == NK - 1),
                    )
                if half in (None, 1):
                    nc.vector.tensor_copy(dst[m][:, ts(c, QC)], p)

            def emit_v(c, sm):
                st = c * 4 + sm
                p = pj_ps.tile([128, QC], F32, tag="pj", name=f"vp{st}")
                for k in range(NK):
                    nc.tensor.matmul(
                        p[:, 0:DPC], xt[k][:, ts(st, 128)], wv_s[k],
                        start=(k == 0), stop=(k == NK - 1),
                    )
                nc.vector.tensor_copy(
                    vt[st][:, :, 0:HD],
                    p[:, 0:DPC].rearrange("p (h d) -> p h d", h=HPC),
                )

            def emit_wo(c, m, half=None):
                key = (c, "wo", m)
                if half in (None, 0):
                    pj_half[key] = pj_ps.tile(
                        [128, QC], F32, tag="pj", name=f"wp{c}_{m}"
                    )
                p = pj_half[key]
                ks = range(NK) if half is None else (
                    range(NK // 2) if half == 0 else range(NK // 2, NK)
                )
                for k in ks:
                    nc.tensor.matmul(
                        p, wo_s[k][:, ts(m, 128)], gq[c][k],
                        start=(k == 0), stop=(k == NK - 1),
                    )
                if half in (None, 1):
                    ow = norm.tile([128, QC], F32, tag="ow", name=f"ow{c}_{m}")
                    nc.vector.tensor_copy(ow, p)
                    nc.sync.dma_start(out=out[ts(m, 128), ts(c, QC)], in_=ow)

            # filler assignment: PE groups interleaved into attn(c)'s stream,
            # keyed by the m-iteration index at whose TOP they are emitted.
            # v(c+?) groups must precede the AV matmuls that read their vt
            # tiles; wo(c) groups must come well after AG(c) is in flight.
            fillers = {
                0: {0: ("v", 0, 0), 1: ("v", 0, 1), 2: ("v", 0, 2), 3: ("v", 0, 3),
                    4: ("qk", 1, "q", 0), 5: ("qk", 1, "q", 1),
                    6: ("qk", 1, "k", 0), 7: ("qk", 1, "k", 1)},
                1: {0: ("v", 1, 0), 1: ("v", 1, 1), 2: ("v", 1, 2), 3: ("v", 1, 3),
                    6: ("qk", 2, "q", 0), 9: ("qk", 2, "q", 1),
                    12: ("qk", 2, "k", 0), 15: ("qk", 2, "k", 1)},
                2: {0: ("v", 2, 0), 1: ("v", 2, 1), 2: ("v", 2, 2), 3: ("v", 2, 3),
                    6: ("qk", 3, "q", 0), 10: ("qk", 3, "q", 1),
                    14: ("qk", 3, "k", 0), 16: ("qk", 3, "k", 1),
                    18: ("wo", 0, 0), 21: ("wo", 0, 1)},
                3: {0: ("v", 3, 0), 1: ("v", 3, 1), 2: ("v", 3, 2), 3: ("v", 3, 3),
                    6: ("wo", 1, 0), 10: ("wo", 1, 1),
                    28: ("wo", 2, 0), 30: ("wo", 2, 1)},
            }

            def emit_filler(f):
                if f[0] == "qk":
                    emit_qk(f[1], f[2], f[3])
                elif f[0] == "v":
                    emit_v(f[1], f[2])
                else:
                    emit_wo(f[1], f[2])

            def attn_chunk(qc):
                fl = dict(fillers[qc])
                n_k = (qc + 1) * 4
                it = 0
                bounce = dram.tile([DPC, QC], BF16, name=f"bounce{qc}")
                for hp in range(2):
                    avA = av_ps.tile([128, QC], F32, tag="avA", name=f"avA{qc}_{hp}")
                    avB = av_ps.tile([128, QC], F32, tag="avB", name=f"avB{qc}_{hp}")
                    for m in range(n_k):
                        if it in fl:
                            emit_filler(fl.pop(it))
                        it += 1
                        # causal: columns j < off are fully masked for this k-tile
                        off = max(0, (m - 4 * qc) * 128)
                        sc = sc_ps.tile(
                            [128, 2 * QC], F32, tag="sc", name=f"sc{qc}_{hp}_{m}"
                        )
                        sc3 = sc[:].rearrange("p (t q) -> p t q", t=2)
                        nc.tensor.matmul(
                            sc[:, off:QC],
                            kT[hp][0:64, ts(m, 128)],
                            qT[hp][0:64, qc * QC + off:(qc + 1) * QC],
                            start=True, stop=True,
                        )
                        nc.tensor.matmul(
                            sc[:, QC + off:2 * QC],
                            kT[hp][64:128, ts(m, 128)],
                            qT[hp][64:128, qc * QC + off:(qc + 1) * QC],
                            start=True, stop=True,
                        )
                        pt = ptile.tile(
                            [128, 2 * QC], BF16, tag="pt", name=f"pt{qc}_{hp}_{m}"
                        )
                        pt3 = pt[:].rearrange("p (t q) -> p t q", t=2)
                        nc.scalar.activation(
                            pt3[:, :, off:QC], sc3[:, :, off:QC],
                            mybir.ActivationFunctionType.Exp,
                            scale=1.0 / np.sqrt(HD),
                        )
                        if m >= 4 * qc:
                            ri = m - 4 * qc
                            m3 = mask_t[ri][:].rearrange("p (t q) -> p t q", t=2)
                            nc.vector.tensor_mul(
                                pt3[:, :, off:QC], pt3[:, :, off:QC],
                                m3[:, :, off:QC],
                            )
                        nc.tensor.matmul(
                            avA[0:HD + 1, off:QC], vt[m][:, 2 * hp, :],
                            pt[:, off:QC],
                            start=(m == 0), stop=(m == n_k - 1),
                        )
                        nc.tensor.matmul(
                            avB[0:HD + 1, off:QC], vt[m][:, 2 * hp + 1, :],
                            pt[:, QC + off:2 * QC],
                            start=(m == 0), stop=(m == n_k - 1),
                        )
                    for hh, av in ((2 * hp, avA), (2 * hp + 1, avB)):
                        # drain av (PSUM) quickly so the next hp's AV can
                        # reuse the bank: rows 0:64 to avs, denom row to a
                        # partition-0 tile for the (partition-aligned) recip
                        avs = norm.tile([HD, QC], F32, tag="avs", name=f"avs{qc}_{hh}")
                        nc.vector.tensor_copy(avs, av[0:HD, :])
                        dn = norm.tile([1, QC], F32, tag="dn", name=f"dn{qc}_{hh}")
                        nc.vector.tensor_copy(dn, av[HD:HD + 1, :])
                        recip = norm.tile([1, QC], F32, tag="recip", name=f"rc{qc}_{hh}")
                        nc.vector.reciprocal_approx_fast(recip, dn)
                        rb = norm.tile([HD, QC], F32, tag="rb", name=f"rb{qc}_{hh}")
                        nc.gpsimd.partition_broadcast(rb, recip, channels=HD)
                        nc.vector.tensor_mul(
                            yT[hh][:, ts(qc, QC)], avs, rb
                        )
                # leftover fillers (shouldn't normally happen)
                for key in sorted(fl):
                    emit_filler(fl.pop(key))

                # ship this chunk's yT through the group AllGather immediately
                for h in range(HPC):
                    nc.sync.dma_start(
                        out=bounce[ts(h, HD), :], in_=yT[h][:, ts(qc, QC)]
                    )
                g = dram.tile([D, QC], BF16, name=f"gath{qc}")
                nc.gpsimd.collective_compute(
                    "AllGather",
                    mybir.AluOpType.bypass,
                    replica_groups=GROUPS,
                    ins=[bounce[:].opt()],
                    outs=[g[:].opt()],
                )
                gath[qc] = g

            def emit_gq_loads(c):
                for k in range(NK):
                    nc.sync.dma_start(out=gq[c][k], in_=gath[c][ts(k, 128), :])

            # ---------- fused pipeline ----------
            for m in range(2):
                emit_qk(0, "q", m)
            for m in range(2):
                emit_qk(0, "k", m)

            attn_chunk(0)
            attn_chunk(1)
            emit_gq_loads(0)        # AG0 done by now; must not block bounce(2)
            attn_chunk(2)
            emit_gq_loads(1)        # AG1 long done
            emit_gq_loads(2)        # waits on AG2 (in flight) on the sync queue
            attn_chunk(3)
            emit_gq_loads(3)        # waits on AG3; nothing left on sync queue
            emit_wo(3, 0)
            emit_wo(3, 1)

    nc.finalize()
    return nc


def make_masks():
    i = np.arange(KT)[:, None]
    j = np.arange(QC)[None, :]
    m = np.zeros((4, KT, 2 * QC), dtype=ml_dtypes.bfloat16)
    for r in range(4):
        half = ((r * KT + i) <= j).astype(ml_dtypes.bfloat16)
        m[r, :, 0:QC] = half
        m[r, :, QC:2 * QC] = half
    return m


def shard_inputs(x, wq, wk, wv, wo):
    """Full f32 inputs -> per-core in_maps (bf16)."""
    bf = ml_dtypes.bfloat16
    masks = make_masks()
    wqT = np.ascontiguousarray(wq.T).astype(bf)
    wkT = np.ascontiguousarray(wk.T).astype(bf)
    wvT = np.ascontiguousarray(wv.T).astype(bf)
    woT = np.ascontiguousarray(wo.T).astype(bf)
    in_maps = []
    for c in range(N_CORES):
        b, tp = divmod(c, TP)
        sl = slice(tp * DPC, (tp + 1) * DPC)
        in_maps.append({
            "xT": np.ascontiguousarray(x[b].T).astype(bf),
            "wqT": np.ascontiguousarray(wqT[:, sl]),
            "wkT": np.ascontiguousarray(wkT[:, sl]),
            "wvT": np.ascontiguousarray(wvT[:, sl]),
            "woT": np.ascontiguousarray(woT[:, sl]),
            "masks": masks,
        })
    return in_maps


def assemble_output(results):
    """Per-core F^T shards [DPC, S] -> full [B, S, D] f32."""
    outs = []
    for b in range(B):
        ft = np.concatenate(
            [results[b * TP + tp]["out"] for tp in range(TP)], axis=0
        )  # [D, S]
        outs.append(ft.T)  # [S, D]
    return np.stack(outs, axis=0)


_NC_CACHE = []


def kernel(x, wq, wk, wv, wo):
    """Full-input distributed attention on 8 NeuronCores; returns full output."""
    x = np.asarray(x, dtype=np.float32)
    wq = np.asarray(wq, dtype=np.float32)
    wk = np.asarray(wk, dtype=np.float32)
    wv = np.asarray(wv, dtype=np.float32)
    wo = np.asarray(wo, dtype=np.float32)
    if not _NC_CACHE:
        _NC_CACHE.append(build())
    nc = _NC_CACHE[0]
    in_maps = shard_inputs(x, wq, wk, wv, wo)
    from concourse import bass2jax
    results = bass2jax.run_bass_via_pjrt(nc, in_maps, n_cores=N_CORES)
    return assemble_output(results).astype(np.float32)
